# revision 48
# baseline (speedup 1.0000x reference)
"""GraphSAGE 2-layer (SAGEConv mean-aggregation) Bass kernel for 8 TRN2 NeuronCores.

v2 design (dst-sharded, slot-ordered, linear-recombined):
  - Destination nodes sharded across 8 cores (12500/core); a greedy balancer
    assigns nodes to 98 windows x 128 slots so each (window, src-block) cell
    has ~512 edges -> near-uniform SPMD schedule (K shared across cores).
  - Aggregation per chunk of 128 edges: dma_gather pulls source rows (bf16)
    from HBM; TensorE accumulates psum[feat, slot] += gt^T @ S where S is a
    pure-0/1 one-hot streamed from DRAM in fp8 (16KB/chunk, full-rate DMA).
    Per window, DVE fuses psum * DINV (resident per-slot inv-degree) -> mean^T.
  - Layer-1 h rows are produced in slot order [slot, feat] directly (no
    transpose, no scatter): h = mean @ Wl1^T + x @ Wr1^T + b1 via three
    matmuls with feat-partition lhsT operands (m1, xT resident in SBUF).
  - h shard (slot-ordered rows) is AllGathered into hfull; layer-2 gather
    blocks are contiguous 2-core row ranges of hfull, which coincide with
    the layer-1 node-id blocks, so the balancer balances both layers.
  - Layer 2 uses linearity: out = m2 @ Wl2^T + m1 @ (Wr2 Wl1)^T
    + x @ (Wr2 Wr1)^T + (b2 + b1 @ Wr2^T), avoiding any need for h^T.
  - Output written in slot order (f32), inverse-permuted on host.
  - Gather-index streams load in 8-call groups; gather calls prefetch one
    call ahead; sval/dinv loads ride the scalar-engine HWDGE queue to keep
    the sync engine clear. The SWDGE descriptor-prep on GPSIMD (~2.3us per
    1024-row gather call) is the throughput bound on TRN2.
"""

import sys

sys.path.insert(0, "/opt/trn_rl_repo")

from contextlib import ExitStack
from dataclasses import dataclass

import ml_dtypes
import numpy as np

import concourse.bacc as bacc
import concourse.bass as bass
import concourse.mybir as mybir
import concourse.tile as tile
from concourse.bass_utils import run_bass_kernel_spmd

BF = mybir.dt.bfloat16
F32 = mybir.dt.float32
I16 = mybir.dt.int16
FP8 = mybir.dt.float8e4
bfnp = ml_dtypes.bfloat16
f8np = ml_dtypes.float8_e4m3


@dataclass
class Cfg:
    N: int = 100000      # total nodes
    D: int = 128         # feature dim
    C: int = 8           # cores
    NB: int = 4          # source blocks (int16 gather index limit)
    WN: int = 98         # windows per core (128 dst nodes each)
    CALL: int = 1024     # gather indices per dma_gather call (ucode max;
                         # 1280/2048 hang the SWDGE ucode)
    IDXG: int = 8        # gather calls per idx-stream load
    SCW: int = 8         # windows per h-shard write
    OCW: int = 8         # windows per final output dma
    SVW: int = 16        # one-hot chunks per sval dma
    PF: int = 0          # layer-2 gather calls per block prepped during AllGather
                         # (nonzero raced/hung on HW: deferred-DMA completion
                         # gating is unreliable in this stack — keep 0)

    @property
    def NSH(self):
        return self.N // self.C

    @property
    def BS(self):
        return self.N // self.NB

    @property
    def SLOTS(self):
        return self.WN * 128

    @property
    def CALLCH(self):
        return self.CALL // 128


CFG = Cfg()
# window-quarter boundaries (multiples of SCW; last takes the remainder)
QW = [0, 24, 48, 72, 98]
ALLGATHER_OVERLAP = False
SVAL_ON_SYNC = False


# ---------------------------------------------------------------- host prep


def _balance_core(dnb, WN, cap=128, ctarget=512):
    """Assign nodes (rows of dnb, per-block in-degree vectors) to WN bins of
    <=cap nodes, aiming for per-(bin, block) sums <= target. Overflow (when a
    block's total exceeds WN*ctarget) is concentrated in the LAST windows.
    Returns (bin id per node, binsum)."""
    nn, NB = dnb.shape
    T = dnb.sum(0)
    target = np.full((WN, NB), ctarget, np.int64)
    for b in range(NB):
        q = max(0, -(-int(T[b] - WN * ctarget) // 128))
        for i in range(min(q, WN)):
            target[WN - 1 - i, b] += 128
    tot = dnb.sum(1)
    order = np.argsort(-tot, kind="stable")
    binsum = np.zeros((WN, NB), np.int64)
    binslots = np.zeros(WN, np.int64)
    assign = np.full(nn, -1, np.int64)
    tgt = target.astype(np.float64)
    for n in order:
        dv = dnb[n]
        fill = ((binsum + dv) / tgt).max(axis=1)
        fill += 1e-5 * binslots
        fill[binslots >= cap] = 1e30
        j = int(np.argmin(fill))
        assign[n] = j
        binsum[j] += dv
        binslots[j] += 1

    # repair: evict small-degree nodes from violated cells into bins with
    # slack (move if a slot is free, else swap with a light partner)
    for _ in range(30):
        viol = np.argwhere(binsum > target)
        if len(viol) == 0:
            break
        moved = 0
        for j, b in viol:
            guard = 0
            while binsum[j, b] > target[j, b] and guard < 64:
                guard += 1
                members = np.where(assign == j)[0]
                mb = dnb[members, b]
                cand_n = members[mb > 0]
                if len(cand_n) == 0:
                    break
                cand_n = cand_n[np.argsort(dnb[cand_n, b], kind="stable")]
                done = False
                for n in cand_n[:8]:
                    dv = dnb[n]
                    ok = ((binsum + dv) <= target).all(axis=1) & (binslots < cap)
                    ok[j] = False
                    cand = np.where(ok)[0]
                    if len(cand):
                        j2 = int(cand[np.argmin(((binsum[cand] + dv) / target[cand]).max(1))])
                        assign[n] = j2
                        binsum[j] -= dv
                        binsum[j2] += dv
                        binslots[j] -= 1
                        binslots[j2] += 1
                        moved += 1
                        done = True
                        break
                    for j2 in np.argsort(binsum[:, b])[:24]:
                        if j2 == j:
                            continue
                        mem2 = np.where(assign == j2)[0]
                        if len(mem2) == 0:
                            continue
                        m = mem2[np.argmin(dnb[mem2, b])]
                        dm = dnb[m]
                        if dm[b] >= dv[b]:
                            continue
                        nj = binsum[j] - dv + dm
                        nj2 = binsum[j2] - dm + dv
                        if (nj <= target[j]).all() and (nj2 <= target[j2]).all():
                            assign[n], assign[m] = j2, j
                            binsum[j] = nj
                            binsum[j2] = nj2
                            moved += 1
                            done = True
                            break
                    if done:
                        break
                if not done:
                    break
        if moved == 0:
            break
    return assign, binsum


def _build_layer_streams(cfg, ew, eb, ep, eloc, counts, K, ncalls):
    """Pack one layer's per-core edge streams (already sorted by (w, b)) into
    gather-index streams (per block) and fp8 one-hot chunk values.
    eloc = block-local gather row per edge. Returns (gidx [128, G], sval fp8
    [128, TCH*128])."""
    NB, WN, CALL = cfg.NB, cfg.WN, cfg.CALL
    TCH = int(K.sum())
    Sb = (K.sum(axis=0) * 128).astype(np.int64)
    gstreams = [np.zeros(int(ncalls[b]) * CALL, np.int16) for b in range(NB)]
    for b in range(NB):
        if Sb[b] < ncalls[b] * CALL:
            gstreams[b][Sb[b]:] = -1  # tail of last call: skipped by DMA
    sval = np.zeros((TCH, 128, 128), f8np)

    eoff = 0
    gcol = 0
    posb = np.zeros(NB, np.int64)
    for w in range(WN):
        for b in range(NB):
            L = int(counts[w, b])
            kwb = int(K[w, b])
            if kwb == 0:
                assert L == 0
                continue
            sl = slice(eoff, eoff + L)
            st = int(posb[b])
            gstreams[b][st:st + L] = eloc[sl].astype(np.int16)
            # pads within the cell keep idx 0 (gather row 0, killed by
            # all-zero one-hot rows)
            ar = np.arange(L)
            sval[gcol + ar // 128, ar % 128, ep[sl]] = 1.0
            posb[b] += kwb * 128
            gcol += kwb
            eoff += L
    assert eoff == ew.shape[0]
    assert gcol == TCH

    gparts = []
    for b in range(NB):
        arr = gstreams[b].reshape(-1, 16).T  # [16, Sb_pad/16]
        gparts.append(arr)
    gidx16 = np.concatenate(gparts, axis=1)
    gidx = np.tile(gidx16, (8, 1)).astype(np.int16)
    svalT = np.ascontiguousarray(sval.transpose(1, 0, 2).reshape(128, -1))
    return gidx, svalT


def prep(x, edge_index, cfg=CFG):
    """Host-side sharding/schedule. Returns (schedule, per-core input maps)."""
    C, NB, WN, NSH, BS, CALL = cfg.C, cfg.NB, cfg.WN, cfg.NSH, cfg.BS, cfg.CALL
    src = np.asarray(edge_index[0]).astype(np.int64)
    dst = np.asarray(edge_index[1]).astype(np.int64)

    deg = np.bincount(dst, minlength=cfg.N).astype(np.float64)
    invdeg = (1.0 / np.maximum(deg, 1.0)).astype(np.float32)

    ecore = dst // NSH
    eblock = src // BS

    # --- per-core balance: node-local id -> (window, pos)
    win_of = np.zeros(cfg.N, np.int64)
    pos_of = np.zeros(cfg.N, np.int64)
    counts1 = np.zeros((C, WN, NB), np.int64)
    for c in range(C):
        lo = c * NSH
        dnb = np.zeros((NSH, NB), np.int64)
        emask = ecore == c
        np.add.at(dnb, (dst[emask] - lo, eblock[emask]), 1)
        assign, binsum = _balance_core(dnb, WN)
        kt = np.ceil(binsum / 128).astype(np.int64)
        key = [tuple(-kt[j]) + tuple(-binsum[j]) for j in range(WN)]
        order = sorted(range(WN), key=lambda j: key[j])
        rank = np.empty(WN, np.int64)
        rank[order] = np.arange(WN)
        w = rank[assign]
        win_of[lo:lo + NSH] = w
        order2 = np.lexsort((np.arange(NSH), w))
        pos = np.zeros(NSH, np.int64)
        pcount = np.zeros(WN, np.int64)
        for m in order2:
            pos[m] = pcount[w[m]]
            pcount[w[m]] += 1
        pos_of[lo:lo + NSH] = pos
        cnt = np.zeros((WN, NB), np.int64)
        np.add.at(cnt, (w[dst[emask] - lo], eblock[emask]), 1)
        counts1[c] = cnt

    K1 = np.ceil(counts1 / 128).astype(np.int64).max(axis=0)  # [WN, NB]
    TCH1 = int(K1.sum())
    Sb1 = (K1.sum(axis=0) * 128).astype(np.int64)
    ncalls1 = np.ceil(Sb1 / CALL).astype(np.int64)
    lastvalid1 = Sb1 - (ncalls1 - 1) * CALL

    # --- layer-2 structure: hfull rows in (core, window, pos) order;
    # gather blocks are contiguous row ranges (2 cores each)
    jcore = src // NSH
    hrow = jcore * cfg.SLOTS + win_of[src] * 128 + pos_of[src]
    HBS = cfg.SLOTS * C // NB
    jq = hrow // HBS
    hloc = hrow - jq * HBS
    counts2 = np.zeros((C, WN, NB), np.int64)
    for c in range(C):
        emask = ecore == c
        cnt = np.zeros((WN, NB), np.int64)
        np.add.at(cnt, (win_of[dst[emask]] - 0, jq[emask]), 1)
        counts2[c] = cnt
    K2 = np.ceil(counts2 / 128).astype(np.int64).max(axis=0)
    TCH2 = int(K2.sum())
    Sb2 = (K2.sum(axis=0) * 128).astype(np.int64)
    ncalls2 = np.ceil(Sb2 / CALL).astype(np.int64)
    lastvalid2 = Sb2 - (ncalls2 - 1) * CALL

    ewin = win_of[dst]
    epos_d = pos_of[dst]

    in_maps = []
    for c in range(C):
        lo = c * NSH
        emask = ecore == c
        es, ed = src[emask], dst[emask]
        ew = ewin[emask]
        ep = epos_d[emask]

        # layer 1 streams: sort by (w, L1 block)
        eb1 = eblock[emask]
        okey = np.lexsort((np.arange(es.shape[0]), eb1, ew))
        eloc1 = (es - eb1 * BS)
        gidx1, sval1 = _build_layer_streams(
            cfg, ew[okey], eb1[okey], ep[okey], eloc1[okey],
            counts1[c], K1, ncalls1)

        # layer 2 streams: sort by (w, quarter-of-src)
        eb2 = jq[emask]
        eloc2 = hloc[emask]
        okey2 = np.lexsort((np.arange(es.shape[0]), eb2, ew))
        gidx2, sval2 = _build_layer_streams(
            cfg, ew[okey2], eb2[okey2], ep[okey2], eloc2[okey2],
            counts2[c], K2, ncalls2)

        # slot -> node-local map (host unshard) + xT / dinv in slot order
        sl_nodes = np.full(cfg.SLOTS, -1, np.int64)
        msk = np.arange(cfg.N)[lo:lo + NSH]
        sl_idx = win_of[msk] * 128 + pos_of[msk]
        sl_nodes[sl_idx] = np.arange(NSH)

        xT = np.zeros((cfg.D, cfg.SLOTS), bfnp)
        xT[:, sl_idx] = np.asarray(x[lo:lo + NSH]).astype(bfnp).T
        dinv_row = np.zeros(cfg.SLOTS, np.float32)
        dinv_row[sl_idx] = invdeg[lo:lo + NSH]
        dinv = np.broadcast_to(dinv_row.astype(bfnp), (128, cfg.SLOTS)).copy()

        in_maps.append(dict(
            gidx1=gidx1, gidx2=gidx2, sval1=sval1, sval2=sval2,
            xT=np.ascontiguousarray(xT), dinv=dinv,
            slot_nodes=sl_nodes,                   # host-only
        ))

    sched = dict(K1=K1, TCH1=TCH1, ncalls1=ncalls1, lastvalid1=lastvalid1,
                 K2=K2, TCH2=TCH2, ncalls2=ncalls2, lastvalid2=lastvalid2)
    return sched, in_maps


# ---------------------------------------------------------------- program


def build(cfg, sched):
    C, D, NB, WN = cfg.C, cfg.D, cfg.NB, cfg.WN
    CALL, CALLCH, SLOTS, SVW = cfg.CALL, cfg.CALLCH, cfg.SLOTS, cfg.SVW
    K = {1: sched["K1"], 2: sched["K2"]}
    TCH = {1: sched["TCH1"], 2: sched["TCH2"]}
    ncalls = {1: sched["ncalls1"], 2: sched["ncalls2"]}
    lastvalid = {1: sched["lastvalid1"], 2: sched["lastvalid2"]}
    GOFF = {}
    for L in (1, 2):
        gc = [int(ncalls[L][b]) * (CALL // 16) for b in range(NB)]
        GOFF[L] = np.concatenate([[0], np.cumsum(gc)]).astype(int)
    qrows = [(QW[q + 1] - QW[q]) * 128 for q in range(4)]

    nc = bacc.Bacc(None, num_devices=C, num_swdge_queues=4,
                   dynamic_dma_scratch_size=32768)
    x_d = nc.dram_tensor("xbf", [cfg.N, D], BF, kind="ExternalInput")
    xT_d = nc.dram_tensor("xT", [D, SLOTS], BF, kind="ExternalInput")
    dinv_d = nc.dram_tensor("dinv", [128, SLOTS], BF, kind="ExternalInput")
    gidx_d = {L: nc.dram_tensor(f"gidx{L}", [128, int(GOFF[L][-1])], I16,
                                kind="ExternalInput") for L in (1, 2)}
    sval_d = {L: nc.dram_tensor(f"sval{L}", [128, TCH[L] * 128], FP8,
                                kind="ExternalInput") for L in (1, 2)}
    w_d = {}
    for nm in ("wlt1", "wrt1", "wlt2", "wr2l1t", "wr2r1t"):
        w_d[nm] = nc.dram_tensor(nm, [D, D], BF, kind="ExternalInput")
    b1_d = nc.dram_tensor("b1r", [1, D], F32, kind="ExternalInput")
    b2_d = nc.dram_tensor("b2e", [1, D], F32, kind="ExternalInput")
    out_d = nc.dram_tensor("out", [SLOTS, D], F32, kind="ExternalOutput")

    ones_d = nc.inline_tensor(np.ones((1, 128), np.float32), "onesc")

    hsh_d = nc.dram_tensor("hsh", [SLOTS, D], BF)
    hfull_d = nc.dram_tensor("hfull", [C * SLOTS, D], BF, addr_space="Shared")
    HBS = SLOTS * C // NB

    with tile.TileContext(nc) as tc, ExitStack() as ctx:
        const = ctx.enter_context(tc.tile_pool(name="const", bufs=1))
        idxp = ctx.enter_context(tc.tile_pool(name="idx", bufs=3))
        gpool = ctx.enter_context(tc.tile_pool(name="gather", bufs=5))
        spool = ctx.enter_context(tc.tile_pool(name="sv", bufs=5))
        m2p = ctx.enter_context(tc.tile_pool(name="m2", bufs=2))
        dvp = ctx.enter_context(tc.tile_pool(name="dv", bufs=3))
        m1p = ctx.enter_context(tc.tile_pool(name="m1", bufs=1))
        xtp = ctx.enter_context(tc.tile_pool(name="xt", bufs=1))
        stgp = ctx.enter_context(tc.tile_pool(name="stg", bufs=2))
        ostgp = ctx.enter_context(tc.tile_pool(name="ostg", bufs=2))
        psA = ctx.enter_context(tc.tile_pool(name="psA", bufs=4, space="PSUM"))
        psB = ctx.enter_context(tc.tile_pool(name="psB", bufs=4, space="PSUM"))

        def load(pool, dram, shape, dtype):
            t = pool.tile(shape, dtype, tag=dram.name)
            nc.sync.dma_start(t[:], dram[:])
            return t

        ones_s = load(const, ones_d, [1, 128], F32)
        w_s = {nm: load(const, w_d[nm], [D, D], BF) for nm in w_d}
        b1_s = load(const, b1_d, [1, D], F32)
        b2_s = load(const, b2_d, [1, D], F32)
        m1_s = m1p.tile([D, SLOTS], BF, tag="m1")
        xT_s = None  # loaded after the first gather calls are issued

        class Stream:
            """Per-layer gather stream state, shared with the prefill path."""

            def __init__(self, L):
                self.L = L
                self.posb = [0] * NB
                self.gt_of = {}
                self.ix_of = {}
                self.issued = set()
                self.pfsem = None

            def src_ap(self, b):
                if self.L == 1:
                    return x_d[b * cfg.BS:(b + 1) * cfg.BS, :]
                return hfull_d[b * HBS:(b + 1) * HBS, :]

            def issue_call(self, b, call_i, prep=False):
                L = self.L
                if call_i >= int(ncalls[L][b]) or (b, call_i) in self.issued:
                    return
                self.issued.add((b, call_i))
                IDXG = cfg.IDXG
                gi, gc = divmod(call_i, IDXG)
                if gc == 0:
                    ncol = min(IDXG * (CALL // 16),
                               (int(ncalls[L][b]) - gi * IDXG) * (CALL // 16))
                    it = idxp.tile([128, IDXG * (CALL // 16)], I16, tag=f"ix{b}",
                                   name=f"ix{L}_{b}_{gi}")
                    ioff = int(GOFF[L][b]) + gi * IDXG * (CALL // 16)
                    nc.sync.dma_start(it[:, :ncol],
                                      gidx_d[L][:, ioff:ioff + ncol])
                    self.ix_of[b] = it
                g = gpool.tile([128, CALLCH, 128], BF, tag=f"g{b}",
                               name=f"g{L}_{b}_{call_i}")
                nvalid = (CALL if call_i < int(ncalls[L][b]) - 1
                          else int(lastvalid[L][b]))
                kw = {}
                if prep:
                    if self.pfsem is None:
                        self.pfsem = [nc.alloc_semaphore(f"pfs{b2}")
                                      for b2 in range(NB)]
                    kw = dict(prepare_only=True, sem=self.pfsem[b],
                              queue_num=b)
                nc.gpsimd.dma_gather(
                    out_ap=g[:], in_ap=self.src_ap(b),
                    idxs_ap=self.ix_of[b][:, gc * (CALL // 16):(gc + 1) * (CALL // 16)],
                    num_idxs=CALL, num_idxs_reg=nvalid, elem_size=D, **kw)
                self.gt_of[(b, call_i)] = g

        def run_layer(L, stream):
            KL = K[L]
            posb = stream.posb
            gt_of = stream.gt_of
            sv_of = {}
            stg_tile = [None]
            ostg_tile = [None]
            gcol = [0]
            SCW, OCW = cfg.SCW, cfg.OCW

            def issue_call(b, call_i):
                stream.issue_call(b, call_i)

            NSV = -(-TCH[L] // SVW)
            SV_DMA_ENGINE = nc.sync.dma_start if SVAL_ON_SYNC else nc.scalar.dma_start

            def issue_sval(si):
                if si >= NSV:
                    return
                nchk = min(SVW, TCH[L] - si * SVW)
                st = spool.tile([128, SVW, 128], FP8, tag="sv",
                                name=f"sv{L}_{si}")
                SV_DMA_ENGINE(
                    st[:, :nchk, :],
                    sval_d[L][:, si * SVW * 128:si * SVW * 128 + nchk * 128])
                sv_of[si] = st

            dv_of = {}

            def issue_dinv(w):
                if w >= WN:
                    return
                dv = dvp.tile([128, 128], BF, tag="dv", name=f"dv{L}_{w}")
                nc.scalar.dma_start(dv[:], dinv_d[:, w * 128:(w + 1) * 128])
                dv_of[w] = dv

            for b in range(NB):
                issue_call(b, 0)
            issue_sval(0)
            issue_dinv(0)
            issue_dinv(1)

            for w in range(WN):
                issue_dinv(w + 2)
                nchunks_w = int(KL[w].sum())
                psum_a = psA.tile([128, 128], F32, tag="agg")
                ci = 0
                for b in range(NB):
                    for k in range(int(KL[w, b])):
                        pos = posb[b]
                        call_i, col = divmod(pos, CALLCH)
                        if col == 0:
                            issue_call(b, call_i + 1)
                        g = gcol[0]
                        si, sc = divmod(g, SVW)
                        if sc == 0:
                            issue_sval(si + 1)
                            sv_of.pop(si - 1, None)
                        nc.tensor.matmul(
                            out=psum_a[:], lhsT=gt_of[(b, call_i)][:, col, :],
                            rhs=sv_of[si][:, sc, :],
                            start=(ci == 0), stop=(ci == nchunks_w - 1),
                        )
                        if col == CALLCH - 1:
                            gt_of.pop((b, call_i), None)
                        gcol[0] += 1
                        posb[b] += 1
                        ci += 1
                wsl = slice(w * 128, (w + 1) * 128)
                if L == 1:
                    m_ap = m1_s[:, wsl]
                else:
                    m2t = m2p.tile([128, 128], BF, tag="m2")
                    m_ap = m2t[:]
                if nchunks_w:
                    nc.vector.tensor_tensor(out=m_ap, in0=psum_a[:],
                                            in1=dv_of[w][:],
                                            op=mybir.AluOpType.mult)
                else:
                    nc.vector.memset(m_ap, 0.0)
                dv_of.pop(w - 1, None)
                psum_h = psB.tile([128, 128], F32, tag="h")
                if L == 1:
                    nc.tensor.matmul(out=psum_h[:], lhsT=m1_s[:, wsl],
                                     rhs=w_s["wlt1"][:], start=True, stop=False)
                    nc.tensor.matmul(out=psum_h[:], lhsT=xT_s[:, wsl],
                                     rhs=w_s["wrt1"][:], start=False, stop=False)
                    nc.tensor.matmul(out=psum_h[:], lhsT=ones_s[0:1, :],
                                     rhs=b1_s[0:1, :], start=False, stop=True)
                    wi = w % SCW
                    if wi == 0:
                        stg_tile[0] = stgp.tile([128, SCW, 128], BF, tag="stg",
                                                name=f"stg{w}")
                    nc.scalar.activation(stg_tile[0][:, wi, :], psum_h[:],
                                         mybir.ActivationFunctionType.Identity)
                    if wi == SCW - 1 or w == WN - 1:
                        used = wi + 1
                        w0 = w - wi
                        oap = hsh_d[:].rearrange("(w p) f -> p w f", p=128)
                        nc.sync.dma_start(oap[:, w0:w0 + used, :],
                                          stg_tile[0][:, :used, :])
                else:
                    nc.tensor.matmul(out=psum_h[:], lhsT=m2t[:],
                                     rhs=w_s["wlt2"][:], start=True, stop=False)
                    nc.tensor.matmul(out=psum_h[:], lhsT=m1_s[:, wsl],
                                     rhs=w_s["wr2l1t"][:], start=False, stop=False)
                    nc.tensor.matmul(out=psum_h[:], lhsT=xT_s[:, wsl],
                                     rhs=w_s["wr2r1t"][:], start=False, stop=False)
                    nc.tensor.matmul(out=psum_h[:], lhsT=ones_s[0:1, :],
                                     rhs=b2_s[0:1, :], start=False, stop=True)
                    wi = w % OCW
                    if wi == 0:
                        ostg_tile[0] = ostgp.tile([128, OCW, 128], F32,
                                                  tag="ostg", name=f"ostg{w}")
                    nc.vector.tensor_copy(ostg_tile[0][:, wi, :], psum_h[:])
                    if wi == OCW - 1 or w == WN - 1:
                        used = wi + 1
                        w0 = w - wi
                        oap = out_d[:].rearrange("(w p) f -> p w f", p=128)
                        nc.sync.dma_start(oap[:, w0:w0 + used, :],
                                          ostg_tile[0][:, :used, :])

        # issue the first gather calls before the 3.2MB xT load so the SWDGE
        # pipeline starts immediately (idx loads not stuck behind xT on sync)
        s1 = Stream(1)
        for _b in range(NB):
            s1.issue_call(_b, 0)
            s1.issue_call(_b, 1)
        xT_s = load(xtp, xT_d, [D, SLOTS], BF)
        run_layer(1, s1)
        nc.gpsimd.collective_compute(
            "AllGather", mybir.AluOpType.bypass,
            replica_groups=[list(range(C))],
            ins=[hsh_d[:]],
            outs=[hfull_d[:]],
        )
        s2 = Stream(2)
        if cfg.PF:
            # prefill: desc-gen for layer-2's first calls runs during the
            # AllGather (data dep deferred to the trigger)
            for b in range(NB):
                for ci in range(cfg.PF):
                    s2.issue_call(b, ci, prep=True)
            for b in range(NB):
                nc.gpsimd.trigger_dma(count=None, queue_num=b)
            # tile's deferred-dep bookkeeping does not gate consumers on the
            # triggered DMA completion: wait for the baked DMA sems (16
            # increments per call) on PE before any consuming matmul
            for b in range(NB):
                nc.tensor.wait_ge(s2.pfsem[b], 16 * min(cfg.PF,
                                                        int(ncalls[2][b])))
        run_layer(2, s2)

    # spread SWDGE gather descriptor generation across the 4 SWDGE queues
    from concourse.tile_sem_assignment import PROC_NAME_TO_IDX
    dmasw0 = PROC_NAME_TO_IDX["DMASW0"]
    for inst in nc.inst_map.values():
        if isinstance(inst, (mybir.InstDMAGatherAnt, mybir.InstDMAScatterAddAnt)):
            if getattr(inst, "gen_mode", 0) == 1:
                continue  # prepare_only: queue must match its trigger
            proc = getattr(inst, "bass_scheduled_proc", None)
            if proc is not None and dmasw0 <= proc < dmasw0 + 8:
                inst.queue_num = (proc - dmasw0) % 4

    nc.compile()
    return nc


# ---------------------------------------------------------------- kernel


def make_shared(inputs, cfg=CFG):
    f64 = np.float64
    Wl1 = np.asarray(inputs["Wl1"], f64)
    Wr1 = np.asarray(inputs["Wr1"], f64)
    Wl2 = np.asarray(inputs["Wl2"], f64)
    Wr2 = np.asarray(inputs["Wr2"], f64)
    b1 = np.asarray(inputs["b1"], f64)
    b2 = np.asarray(inputs["b2"], f64)
    x = np.asarray(inputs["x"], np.float32)

    def tbf(a):
        return np.ascontiguousarray(a.T.astype(np.float32).astype(bfnp))

    return dict(
        xbf=x.astype(bfnp),
        wlt1=tbf(Wl1), wrt1=tbf(Wr1), wlt2=tbf(Wl2),
        wr2l1t=tbf(Wr2 @ Wl1), wr2r1t=tbf(Wr2 @ Wr1),
        b1r=b1.astype(np.float32).reshape(1, cfg.D).copy(),
        b2e=(b2 + Wr2 @ b1).astype(np.float32).reshape(1, cfg.D).copy(),
    )


def unshard(res, slot_nodes, cfg=CFG):
    out = np.empty((cfg.N, cfg.D), np.float32)
    for c in range(cfg.C):
        oc = res.results[c]["out"]
        sn = slot_nodes[c]
        real = sn >= 0
        out[c * cfg.NSH + sn[real]] = oc[real]
    return out


def kernel(**inputs):
    cfg = CFG
    x = np.asarray(inputs["x"], np.float32)
    ei = np.asarray(inputs["edge_index"])
    sched, in_maps = prep(x, ei, cfg)
    nc = build(cfg, sched)
    shared = make_shared(inputs, cfg)
    slot_nodes = [m.pop("slot_nodes") for m in in_maps]
    run_maps = [dict(shared, **m) for m in in_maps]
    res = run_bass_kernel_spmd(nc, run_maps, core_ids=list(range(cfg.C)))
    return unshard(res, slot_nodes, cfg)


if __name__ == "__main__":
    d = np.load("/tmp/inputs.npz")
    ins = {k: d[k] for k in ("x", "edge_index", "Wl1", "Wr1", "b1", "Wl2", "Wr2", "b2")}
    got = kernel(**ins)
    exp = d["expected"]
    err = np.abs(got - exp).max() / np.abs(exp).max()
    print("Relative error:", err)
